# revision 12
# baseline (speedup 1.0000x reference)
"""AttentionGNNLayer on 8 TRN2 NeuronCores (Bass/Tile), v2.

Src-sharded, collective-free; see NOTES.md.  Key points:
- EE edges reduced on host: SW[s] = sum ee_weight over ee_src==s; device
  adds SW[s]*(M[s]+b_n).
- Phase A: bf16 tables via PE: PM[n]=emb@[W1|Wn] (all nodes), Q=emb@W2+b
  (local).  b_n/b_self folded into phase C.  Phase A2: local M/SELF in f32
  (error control: SW amplifies M's bf16 noise otherwise).
- Phase B: 36 super-tiles x 4096 slots, quarter-major (PM split in 4
  int16-addressable quarters of 25024 rows, per-(st,q) runs padded to
  1024).  All dma_gather/dma_scatter_add calls use num_idxs=1024 (HW
  crashes at >=1152; 512 verified, 1024 probed).  x=P+Q; t=tanh(x);
  e=t@w0; g=exp(e+b0); scatter-add [g*M | g] (129 f32, stride 768B) into
  U by src.  Uniqueness per scatter: group-major dealing (max deg 28<36).
- Phase C: out = tanh(SELF + (U+Z*b_n)/(Z+1e-9) + SW*(M+b_n) + b_self).
"""
import os
import sys
from contextlib import ExitStack

import numpy as np

sys.path.insert(0, "/opt/trn_rl_repo")

import ml_dtypes  # noqa: E402

import concourse.tile as tile  # noqa: E402
from concourse import bacc, mybir  # noqa: E402
from concourse.bass_utils import run_bass_kernel_spmd  # noqa: E402
from concourse.masks import make_identity  # noqa: E402

dt = mybir.dt
bf16 = ml_dtypes.bfloat16

N = 100_000
D = 128
NC = 8
SL = N // NC
NB_ALL = 782
NPAD = NB_ALL * 128     # 100096
QR = NPAD // 4          # 25024 rows per PM quarter
NB_SL = 98
QV = NB_SL * 128 + 128  # 12672
TRASH = NB_SL * 128     # 12544
UW = 192                # U row stride (f32, 768B)

NST = 36                # super-tiles / dealing groups (max deg 28 < 36)
WQ = 1024               # slots per (st, quarter) — HW-safe dma_gather size
W = 4 * WQ              # 4096 slots per super-tile
KC = W // 128           # 32
AB = 8
CB = 7
P = 128

_CACHE = {}


def _build():
    if "nc" in _CACHE:
        return _CACHE["nc"]
    nc = bacc.Bacc("TRN2", target_bir_lowering=False, debug=False, num_devices=NC)

    embb = nc.dram_tensor("embb", [NPAD, D], dt.bfloat16, kind="ExternalInput")
    wallb = nc.dram_tensor("wallb", [D, 512], dt.bfloat16, kind="ExternalInput")
    qbias = nc.dram_tensor("qbias", [P, D], dt.float32, kind="ExternalInput")
    bnrep = nc.dram_tensor("bnrep", [P, D], dt.float32, kind="ExternalInput")
    bsrep = nc.dram_tensor("bsrep", [P, D], dt.float32, kind="ExternalInput")
    w0r = nc.dram_tensor("w0r", [P, D], dt.bfloat16, kind="ExternalInput")
    b0c = nc.dram_tensor("b0c", [P, 1], dt.float32, kind="ExternalInput")
    swt_d = nc.dram_tensor("swt", [P, NB_SL], dt.float32, kind="ExternalInput")
    emb32 = nc.dram_tensor("emb32", [NB_SL * P, D], dt.float32,
                           kind="ExternalInput")
    wallms = nc.dram_tensor("wallms", [D, 256], dt.float32,
                            kind="ExternalInput")
    erdq_d = nc.dram_tensor("erdq", [NST, P, W // 16], dt.int16,
                            kind="ExternalInput")
    srci_d = nc.dram_tensor("srci", [NST, P, W // 16], dt.int16,
                            kind="ExternalInput")

    out = nc.dram_tensor("out", [NB_SL * P, D], dt.float32, kind="ExternalOutput")

    pm_d = nc.dram_tensor("pm_d", [NPAD, 256], dt.bfloat16)
    q_d = nc.dram_tensor("q_d", [QV, D], dt.bfloat16)
    m32_d = nc.dram_tensor("m32_d", [NB_SL * 128, D], dt.float32)
    s32_d = nc.dram_tensor("s32_d", [NB_SL * 128, D], dt.float32)
    u_d = nc.dram_tensor("u_d", [QV, UW], dt.float32)

    with tile.TileContext(nc) as tc, ExitStack() as ctx:
        cpool = ctx.enter_context(tc.tile_pool(name="const", bufs=1))

        ident = cpool.tile([P, P], dt.bfloat16)
        make_identity(nc, ident[:])
        ident32 = cpool.tile([P, P], dt.float32)
        make_identity(nc, ident32[:])
        wms_t = cpool.tile([P, 256], dt.float32)
        nc.sync.dma_start(out=wms_t[:], in_=wallms[:, :])
        wall_t = cpool.tile([P, 512], dt.bfloat16)
        nc.sync.dma_start(out=wall_t[:], in_=wallb[:, :])
        qb_t = cpool.tile([P, D], dt.float32)
        nc.sync.dma_start(out=qb_t[:], in_=qbias[:, :])
        bn_t = cpool.tile([P, D], dt.float32)
        nc.sync.dma_start(out=bn_t[:], in_=bnrep[:, :])
        bs_t = cpool.tile([P, D], dt.float32)
        nc.sync.dma_start(out=bs_t[:], in_=bsrep[:, :])
        w0_t = cpool.tile([P, D], dt.bfloat16)
        nc.sync.dma_start(out=w0_t[:], in_=w0r[:, :])
        b0_t = cpool.tile([P, 1], dt.float32)
        nc.sync.dma_start(out=b0_t[:], in_=b0c[:, :])
        sw_t = cpool.tile([P, NB_SL], dt.float32)
        nc.sync.dma_start(out=sw_t[:], in_=swt_d[:, :])

        # zero U table (and Q trash block), 9 blocks per DMA
        zt = cpool.tile([P, 9 * UW], dt.float32)
        nc.vector.memset(zt[:], 0.0)
        z3 = zt[:].rearrange("p (b c) -> p b c", c=UW)
        for b in range(0, QV // P, 9):
            nc.sync.dma_start(
                out=u_d.ap()[b * P:(b + 9) * P, :].rearrange(
                    "(b p) c -> p b c", p=P),
                in_=z3)
        ztb = cpool.tile([P, D], dt.bfloat16)
        nc.vector.memset(ztb[:], 0.0)
        nc.sync.dma_start(out=q_d.ap()[TRASH:TRASH + P, :], in_=ztb[:])

        # ---------------- phase A: bf16 tables ----------------
        with tc.tile_pool(name="pa", bufs=3) as pa, \
             tc.tile_pool(name="paps", bufs=3, space="PSUM") as paps:
            for bb in range(0, NB_ALL, AB):
                nb = min(AB, NB_ALL - bb)
                qn = max(0, min(nb, NB_SL - bb))
                eb = pa.tile([P, AB * D], dt.bfloat16, tag="eb")
                nc.sync.dma_start(
                    out=eb[:, :nb * D],
                    in_=embb[bb * P:(bb + nb) * P, :].rearrange(
                        "(b p) d -> p b d", p=P))
                tabs = pa.tile([P, AB * 256], dt.bfloat16, tag="tabs")
                tabs3 = tabs[:].rearrange("p (b c) -> p b c", c=256)
                qs3 = None
                if qn > 0:
                    qs = pa.tile([P, AB * D], dt.bfloat16, tag="qs")
                    qs3 = qs[:].rearrange("p (b c) -> p b c", c=D)
                for b in range(nb):
                    ebT_ps = paps.tile([P, P], dt.bfloat16, tag="ebT")
                    nc.tensor.transpose(out=ebT_ps[:],
                                        in_=eb[:, b * D:(b + 1) * D],
                                        identity=ident[:])
                    ebT = pa.tile([P, P], dt.bfloat16, tag="ebTs")
                    nc.scalar.copy(out=ebT[:], in_=ebT_ps[:])
                    w = 384 if b < qn else 256
                    mm = paps.tile([P, 512], dt.float32, tag="mm")
                    nc.tensor.matmul(out=mm[:, :w], lhsT=ebT[:],
                                     rhs=wall_t[:, :w], start=True, stop=True)
                    if b % 2 == 0:
                        nc.scalar.copy(out=tabs3[:, b, :], in_=mm[:, 0:256])
                    else:
                        nc.vector.tensor_copy(out=tabs3[:, b, :], in_=mm[:, 0:256])
                    if b < qn:
                        nc.vector.tensor_add(out=qs3[:, b, :],
                                             in0=mm[:, 256:384], in1=qb_t[:])
                nc.sync.dma_start(
                    out=pm_d.ap()[bb * P:(bb + nb) * P, :].rearrange(
                        "(b p) c -> p b c", p=P),
                    in_=tabs3[:, :nb, :])
                if qn > 0:
                    nc.sync.dma_start(
                        out=q_d.ap()[bb * P:(bb + qn) * P, :].rearrange(
                            "(b p) c -> p b c", p=P),
                        in_=qs3[:, :qn, :])

        # ------------- phase A2: local M/SELF tables in f32 -------------
        with tc.tile_pool(name="pa2", bufs=3) as pa2, \
             tc.tile_pool(name="pa2ps", bufs=3, space="PSUM") as pa2ps:
            for bb in range(0, NB_SL, AB):
                nb = min(AB, NB_SL - bb)
                eb2 = pa2.tile([P, AB * D], dt.float32, tag="eb2")
                nc.sync.dma_start(
                    out=eb2[:, :nb * D],
                    in_=emb32[bb * P:(bb + nb) * P, :].rearrange(
                        "(b p) d -> p b d", p=P))
                ms = pa2.tile([P, AB * 256], dt.float32, tag="ms")
                ms3 = ms[:].rearrange("p (b c) -> p b c", c=256)
                for b in range(nb):
                    eT_ps = pa2ps.tile([P, P], dt.float32, tag="eT2")
                    nc.tensor.transpose(out=eT_ps[:],
                                        in_=eb2[:, b * D:(b + 1) * D],
                                        identity=ident32[:])
                    eT = pa2.tile([P, P], dt.float32, tag="eT2s")
                    nc.scalar.copy(out=eT[:], in_=eT_ps[:])
                    mm2 = pa2ps.tile([P, 256], dt.float32, tag="mm2")
                    nc.tensor.matmul(out=mm2[:], lhsT=eT[:], rhs=wms_t[:],
                                     start=True, stop=True)
                    if b % 2 == 0:
                        nc.scalar.copy(out=ms3[:, b, :], in_=mm2[:])
                    else:
                        nc.vector.tensor_copy(out=ms3[:, b, :], in_=mm2[:])
                nc.sync.dma_start(
                    out=m32_d.ap()[bb * P:(bb + nb) * P, :].rearrange(
                        "(b p) c -> p b c", p=P),
                    in_=ms3[:, :nb, 0:D])
                nc.sync.dma_start(
                    out=s32_d.ap()[bb * P:(bb + nb) * P, :].rearrange(
                        "(b p) c -> p b c", p=P),
                    in_=ms3[:, :nb, D:256])

        # ---------------- phase B: edges ----------------
        with tc.tile_pool(name="pb", bufs=2) as pb:
            for st in range(NST):
                dq = pb.tile([P, W // 16], dt.int16, tag="dq")
                nc.sync.dma_start(out=dq[:], in_=erdq_d[st])
                si = pb.tile([P, W // 16], dt.int16, tag="si")
                nc.sync.dma_start(out=si[:], in_=srci_d[st])

                pmt = pb.tile([P, KC * 256], dt.bfloat16, tag="pmt")
                pm3 = pmt[:].rearrange("p (k d) -> p k d", d=256)
                qt = pb.tile([P, KC * D], dt.bfloat16, tag="qt")
                q3 = qt[:].rearrange("p (k d) -> p k d", d=D)
                wc = WQ // P   # 8 payload cols per quarter-run
                for q in range(4):
                    nc.gpsimd.dma_gather(
                        out_ap=pm3[:, q * wc:(q + 1) * wc, :],
                        in_ap=pm_d.ap()[q * QR:(q + 1) * QR, :],
                        idxs_ap=dq[:, q * (WQ // 16):(q + 1) * (WQ // 16)],
                        num_idxs=WQ, num_idxs_reg=WQ, elem_size=256,
                        single_packet=False)
                    nc.gpsimd.dma_gather(
                        out_ap=q3[:, q * wc:(q + 1) * wc, :],
                        in_ap=q_d.ap(),
                        idxs_ap=si[:, q * (WQ // 16):(q + 1) * (WQ // 16)],
                        num_idxs=WQ, num_idxs_reg=WQ, elem_size=D,
                        single_packet=False)

                nc.vector.tensor_add(out=q3, in0=pm3[:, :, 0:D], in1=q3)
                nc.scalar.activation(out=q3, in_=q3,
                                     func=mybir.ActivationFunctionType.Tanh)
                w03 = w0_t[:].rearrange("p (o d) -> p o d", o=1).to_broadcast(
                    [P, KC, D])
                nc.vector.tensor_mul(out=q3, in0=q3, in1=w03)
                et = pb.tile([P, KC], dt.float32, tag="et")
                nc.vector.reduce_sum(out=et[:], in_=q3, axis=mybir.AxisListType.X)
                gt = pb.tile([P, KC], dt.float32, tag="gt")
                nc.scalar.activation(out=gt[:], in_=et[:],
                                     func=mybir.ActivationFunctionType.Exp,
                                     bias=b0_t[:])
                gtb = pb.tile([P, KC], dt.bfloat16, tag="gtb")
                nc.vector.tensor_copy(out=gtb[:], in_=gt[:])
                uin = pb.tile([P, KC * 129], dt.float32, tag="uin")
                u3 = uin[:].rearrange("p (k d) -> p k d", d=129)
                g3 = gtb[:].rearrange("p (k o) -> p k o", o=1).to_broadcast(
                    [P, KC, D])
                nc.vector.tensor_mul(out=u3[:, :, 0:D], in0=pm3[:, :, D:256],
                                     in1=g3)
                nc.vector.tensor_copy(out=u3[:, :, D:D + 1],
                                      in_=gt[:].rearrange("p (k o) -> p k o", o=1))
                for q in range(4):
                    nc.gpsimd.dma_scatter_add(
                        out_ap=u_d.ap()[:, 0:129],
                        in_ap=u3[:, q * wc:(q + 1) * wc, :],
                        idxs_ap=si[:, q * (WQ // 16):(q + 1) * (WQ // 16)],
                        num_idxs=WQ, num_idxs_reg=WQ, elem_size=129,
                        elem_step=UW)

        # ---------------- phase C: combine ----------------
        with tc.tile_pool(name="pc", bufs=3) as pc:
            for bb in range(0, NB_SL, CB):
                ut = pc.tile([P, CB * 129], dt.float32, tag="ut")
                nc.sync.dma_start(
                    out=ut[:],
                    in_=u_d.ap()[bb * P:(bb + CB) * P, 0:129].rearrange(
                        "(b p) c -> p b c", p=P))
                mt = pc.tile([P, CB * D], dt.float32, tag="mt")
                nc.sync.dma_start(
                    out=mt[:],
                    in_=m32_d.ap()[bb * P:(bb + CB) * P, :].rearrange(
                        "(b p) c -> p b c", p=P))
                sf = pc.tile([P, CB * D], dt.float32, tag="sf")
                nc.sync.dma_start(
                    out=sf[:],
                    in_=s32_d.ap()[bb * P:(bb + CB) * P, :].rearrange(
                        "(b p) c -> p b c", p=P))
                ut3 = ut[:].rearrange("p (b c) -> p b c", c=129)
                mt3 = mt[:].rearrange("p (b c) -> p b c", c=D)
                sf3 = sf[:].rearrange("p (b c) -> p b c", c=D)
                ob = pc.tile([P, CB * D], dt.float32, tag="ob")
                ob3 = ob[:].rearrange("p (b c) -> p b c", c=D)
                for b in range(CB):
                    blk = bb + b
                    zp = pc.tile([P, 1], dt.float32, tag="zp")
                    nc.vector.tensor_scalar_add(zp[:], ut3[:, b, D:D + 1], 1e-9)
                    rr = pc.tile([P, 1], dt.float32, tag="rr")
                    nc.vector.reciprocal(rr[:], zp[:])
                    zr = pc.tile([P, 1], dt.float32, tag="zr")
                    nc.vector.tensor_mul(out=zr[:], in0=ut3[:, b, D:D + 1],
                                         in1=rr[:])
                    s1 = pc.tile([P, D], dt.float32, tag="s1")
                    nc.vector.tensor_scalar_mul(s1[:], ut3[:, b, 0:D], rr[:])
                    nc.vector.scalar_tensor_tensor(
                        out=s1[:], in0=bn_t[:], scalar=zr[:], in1=s1[:],
                        op0=mybir.AluOpType.mult, op1=mybir.AluOpType.add)
                    t1 = pc.tile([P, D], dt.float32, tag="t1")
                    nc.vector.tensor_add(out=t1[:], in0=mt3[:, b, :], in1=bn_t[:])
                    nc.vector.scalar_tensor_tensor(
                        out=s1[:], in0=t1[:], scalar=sw_t[:, blk:blk + 1],
                        in1=s1[:], op0=mybir.AluOpType.mult,
                        op1=mybir.AluOpType.add)
                    nc.vector.tensor_add(out=s1[:], in0=s1[:], in1=sf3[:, b, :])
                    nc.vector.tensor_add(out=s1[:], in0=s1[:], in1=bs_t[:])
                    nc.scalar.activation(out=ob3[:, b, :], in_=s1[:],
                                         func=mybir.ActivationFunctionType.Tanh)
                nc.sync.dma_start(
                    out=out.ap()[bb * P:(bb + CB) * P, :].rearrange(
                        "(b p) c -> p b c", p=P),
                    in_=ob3)

    nc.compile()
    _CACHE["nc"] = nc
    return nc


def _wrap16(flat, width):
    """Pack flat int16 idx run into [128, width]: idx j at (j%16, j//16),
    replicated across the 8 16-partition groups."""
    n = len(flat)
    assert n % 16 == 0 and n // 16 <= width, (n, width)
    a = np.zeros((16, width), dtype=np.int16)
    a[:, :n // 16] = flat.reshape(-1, 16).T
    return np.tile(a, (8, 1))


def kernel(node_emb, er_src, er_dst, ee_src, ee_dst, ee_weight,
           W_attn_w, W_attn_b, w0_w, w0_b, W_self_w, W_self_b,
           W_neigh_w, W_neigh_b, **_):
    node_emb = np.asarray(node_emb, np.float32)
    er_src = np.asarray(er_src).astype(np.int64)
    er_dst = np.asarray(er_dst).astype(np.int64)
    ee_src = np.asarray(ee_src).astype(np.int64)
    ee_weight = np.asarray(ee_weight, np.float32)

    wallb = np.concatenate([
        np.asarray(W_attn_w, np.float32)[:D],
        np.asarray(W_neigh_w, np.float32),
        np.asarray(W_attn_w, np.float32)[D:],
        np.asarray(W_self_w, np.float32)], axis=1).astype(bf16)
    qbias = np.broadcast_to(np.asarray(W_attn_b, np.float32), (P, D)).copy()
    bnrep = np.broadcast_to(np.asarray(W_neigh_b, np.float32), (P, D)).copy()
    bsrep = np.broadcast_to(np.asarray(W_self_b, np.float32), (P, D)).copy()
    w0rep = np.broadcast_to(np.asarray(w0_w, np.float32), (P, D)).astype(bf16)
    b0c = np.full((P, 1), float(np.asarray(w0_b)), np.float32)
    wallms = np.concatenate([np.asarray(W_neigh_w, np.float32),
                             np.asarray(W_self_w, np.float32)], axis=1)

    in_maps = []
    for c in range(NC):
        lo = c * SL
        emb_rot32 = np.concatenate([
            np.roll(node_emb, -lo, axis=0),
            np.zeros((NPAD - N, D), np.float32)], axis=0)
        emb_rot = emb_rot32.astype(bf16)

        sel = (er_src >= lo) & (er_src < lo + SL)
        s_loc = (er_src[sel] - lo).astype(np.int32)
        d_rot = ((er_dst[sel] - lo) % N).astype(np.int32)
        o = np.argsort(s_loc, kind="stable")
        s_loc, d_rot = s_loc[o], d_rot[o]
        deg = np.bincount(s_loc, minlength=SL).max()
        assert deg <= NST, deg

        erdq = np.zeros((NST, P, W // 16), np.int16)
        srci = np.zeros((NST, P, W // 16), np.int16)
        i = np.arange(len(s_loc))
        for st in range(NST):
            g = i[i % NST == st]
            dg, sg = d_rot[g], s_loc[g]
            qid = dg // QR
            dflat = np.zeros(W, np.int16)
            sflat = np.full(W, TRASH, np.int16)
            for q in range(4):
                m = qid == q
                cnt = int(m.sum())
                assert cnt <= WQ, (st, q, cnt)
                dflat[q * WQ:q * WQ + cnt] = (dg[m] - q * QR).astype(np.int16)
                sflat[q * WQ:q * WQ + cnt] = sg[m].astype(np.int16)
            erdq[st] = _wrap16(dflat, W // 16)
            srci[st] = _wrap16(sflat, W // 16)

        esel = (ee_src >= lo) & (ee_src < lo + SL)
        es_loc = (ee_src[esel] - lo).astype(np.int64)
        sw = np.bincount(es_loc, weights=ee_weight[esel],
                         minlength=NB_SL * P).astype(np.float32)
        swt = sw.reshape(NB_SL, P).T.copy()

        in_maps.append({
            "embb": emb_rot, "wallb": wallb, "qbias": qbias, "bnrep": bnrep,
            "bsrep": bsrep, "w0r": w0rep, "b0c": b0c, "swt": swt,
            "emb32": emb_rot32[:NB_SL * P], "wallms": wallms,
            "erdq": erdq, "srci": srci,
        })

    nc = _build()
    res = run_bass_kernel_spmd(nc, in_maps, core_ids=list(range(NC)),
                               tmpdir=os.environ.get("BASS_TMPDIR"))
    globals()["LAST"] = res
    return np.concatenate([res.results[c]["out"][:SL] for c in range(NC)], axis=0)


# revision 14
# speedup vs baseline: 1.5051x; 1.5051x over previous
"""AttentionGNNLayer on 8 TRN2 NeuronCores (Bass/Tile), v2.

Src-sharded, collective-free; see NOTES.md.  Key points:
- EE edges reduced on host: SW[s] = sum ee_weight over ee_src==s; device
  adds SW[s]*(M[s]+b_n).
- Phase A: bf16 tables via PE: PM[n]=emb@[W1|Wn] (all nodes), Q=emb@W2+b
  (local).  b_n/b_self folded into phase C.  Phase A2: local M/SELF in f32
  (error control: SW amplifies M's bf16 noise otherwise).
- Phase B: 36 super-tiles x 4096 slots, quarter-major (PM split in 4
  int16-addressable quarters of 25024 rows, per-(st,q) runs padded to
  1024).  All dma_gather/dma_scatter_add calls use num_idxs=1024 (HW
  crashes at >=1152; 512 verified, 1024 probed).  x=P+Q; t=tanh(x);
  e=t@w0; g=exp(e+b0); scatter-add [g*M | g] (129 f32, stride 768B) into
  U by src.  Uniqueness per scatter: group-major dealing (max deg 28<36).
- Phase C: out = tanh(SELF + (U+Z*b_n)/(Z+1e-9) + SW*(M+b_n) + b_self).
"""
import os
import sys
from contextlib import ExitStack

import numpy as np

sys.path.insert(0, "/opt/trn_rl_repo")

import ml_dtypes  # noqa: E402

import concourse.tile as tile  # noqa: E402
from concourse import bacc, mybir  # noqa: E402
from concourse.bass_utils import run_bass_kernel_spmd  # noqa: E402
from concourse.masks import make_identity  # noqa: E402

dt = mybir.dt
bf16 = ml_dtypes.bfloat16

N = 100_000
D = 128
NC = 8
SL = N // NC
NB_ALL = 782
NPAD = NB_ALL * 128     # 100096
QR = NPAD // 4          # 25024 rows per PM quarter
NB_SL = 98
QV = NB_SL * 128 + 128  # 12672
TRASH = NB_SL * 128     # 12544
UW = 192                # U row stride (f32, 768B)

NST = 36                # super-tiles / dealing groups (max deg 28 < 36)
WQ = 1024               # slots per (st, quarter) — HW-safe dma_gather size
W = 4 * WQ              # 4096 slots per super-tile
KC = W // 128           # 32
AB = 8
CB = 7
P = 128

_CACHE = {}


def _build():
    if "nc" in _CACHE:
        return _CACHE["nc"]
    nc = bacc.Bacc("TRN2", target_bir_lowering=False, debug=False, num_devices=NC,
                   num_swdge_queues=4)

    embb = nc.dram_tensor("embb", [NPAD, D], dt.bfloat16, kind="ExternalInput")
    wallb = nc.dram_tensor("wallb", [D, 512], dt.bfloat16, kind="ExternalInput")
    qbias = nc.dram_tensor("qbias", [P, D], dt.float32, kind="ExternalInput")
    bnrep = nc.dram_tensor("bnrep", [P, D], dt.float32, kind="ExternalInput")
    bsrep = nc.dram_tensor("bsrep", [P, D], dt.float32, kind="ExternalInput")
    w0r = nc.dram_tensor("w0r", [P, D], dt.bfloat16, kind="ExternalInput")
    b0c = nc.dram_tensor("b0c", [P, 1], dt.float32, kind="ExternalInput")
    swt_d = nc.dram_tensor("swt", [P, NB_SL], dt.float32, kind="ExternalInput")
    emb32 = nc.dram_tensor("emb32", [NB_SL * P, D], dt.float32,
                           kind="ExternalInput")
    wallms = nc.dram_tensor("wallms", [D, 256], dt.float32,
                            kind="ExternalInput")
    erdq_d = nc.dram_tensor("erdq", [NST, P, W // 16], dt.int16,
                            kind="ExternalInput")
    srci_d = nc.dram_tensor("srci", [NST, P, W // 16], dt.int16,
                            kind="ExternalInput")

    out = nc.dram_tensor("out", [NB_SL * P, D], dt.float32, kind="ExternalOutput")

    pm_d = nc.dram_tensor("pm_d", [NPAD, 256], dt.bfloat16)
    q_d = nc.dram_tensor("q_d", [QV, D], dt.bfloat16)
    m32_d = nc.dram_tensor("m32_d", [NB_SL * 128, D], dt.float32)
    s32_d = nc.dram_tensor("s32_d", [NB_SL * 128, D], dt.float32)
    u_d = nc.dram_tensor("u_d", [QV, UW], dt.float32)

    with tile.TileContext(nc) as tc, ExitStack() as ctx:
        cpool = ctx.enter_context(tc.tile_pool(name="const", bufs=1))

        ident = cpool.tile([P, P], dt.bfloat16)
        make_identity(nc, ident[:])
        ident32 = cpool.tile([P, P], dt.float32)
        make_identity(nc, ident32[:])
        wms_t = cpool.tile([P, 256], dt.float32)
        nc.sync.dma_start(out=wms_t[:], in_=wallms[:, :])
        wall_t = cpool.tile([P, 512], dt.bfloat16)
        nc.sync.dma_start(out=wall_t[:], in_=wallb[:, :])
        qb_t = cpool.tile([P, D], dt.float32)
        nc.sync.dma_start(out=qb_t[:], in_=qbias[:, :])
        bn_t = cpool.tile([P, D], dt.float32)
        nc.sync.dma_start(out=bn_t[:], in_=bnrep[:, :])
        bs_t = cpool.tile([P, D], dt.float32)
        nc.sync.dma_start(out=bs_t[:], in_=bsrep[:, :])
        w0_t = cpool.tile([P, D], dt.bfloat16)
        nc.sync.dma_start(out=w0_t[:], in_=w0r[:, :])
        b0_t = cpool.tile([P, 1], dt.float32)
        nc.sync.dma_start(out=b0_t[:], in_=b0c[:, :])
        sw_t = cpool.tile([P, NB_SL], dt.float32)
        nc.sync.dma_start(out=sw_t[:], in_=swt_d[:, :])

        # zero U table (and Q trash block), 9 blocks per DMA
        zt = cpool.tile([P, 9 * UW], dt.float32)
        nc.vector.memset(zt[:], 0.0)
        z3 = zt[:].rearrange("p (b c) -> p b c", c=UW)
        for b in range(0, QV // P, 9):
            nc.sync.dma_start(
                out=u_d.ap()[b * P:(b + 9) * P, :].rearrange(
                    "(b p) c -> p b c", p=P),
                in_=z3)
        ztb = cpool.tile([P, D], dt.bfloat16)
        nc.vector.memset(ztb[:], 0.0)
        nc.sync.dma_start(out=q_d.ap()[TRASH:TRASH + P, :], in_=ztb[:])

        # ---------------- phase A: bf16 tables ----------------
        with tc.tile_pool(name="pa", bufs=3) as pa, \
             tc.tile_pool(name="paps", bufs=3, space="PSUM") as paps:
            for bb in range(0, NB_ALL, AB):
                nb = min(AB, NB_ALL - bb)
                qn = max(0, min(nb, NB_SL - bb))
                eb = pa.tile([P, AB * D], dt.bfloat16, tag="eb")
                nc.sync.dma_start(
                    out=eb[:, :nb * D],
                    in_=embb[bb * P:(bb + nb) * P, :].rearrange(
                        "(b p) d -> p b d", p=P))
                tabs = pa.tile([P, AB * 256], dt.bfloat16, tag="tabs")
                tabs3 = tabs[:].rearrange("p (b c) -> p b c", c=256)
                qs3 = None
                if qn > 0:
                    qs = pa.tile([P, AB * D], dt.bfloat16, tag="qs")
                    qs3 = qs[:].rearrange("p (b c) -> p b c", c=D)
                for b in range(nb):
                    ebT_ps = paps.tile([P, P], dt.bfloat16, tag="ebT")
                    nc.tensor.transpose(out=ebT_ps[:],
                                        in_=eb[:, b * D:(b + 1) * D],
                                        identity=ident[:])
                    ebT = pa.tile([P, P], dt.bfloat16, tag="ebTs")
                    nc.scalar.copy(out=ebT[:], in_=ebT_ps[:])
                    w = 384 if b < qn else 256
                    mm = paps.tile([P, 512], dt.float32, tag="mm")
                    nc.tensor.matmul(out=mm[:, :w], lhsT=ebT[:],
                                     rhs=wall_t[:, :w], start=True, stop=True)
                    if b % 2 == 0:
                        nc.scalar.copy(out=tabs3[:, b, :], in_=mm[:, 0:256])
                    else:
                        nc.vector.tensor_copy(out=tabs3[:, b, :], in_=mm[:, 0:256])
                    if b < qn:
                        nc.vector.tensor_add(out=qs3[:, b, :],
                                             in0=mm[:, 256:384], in1=qb_t[:])
                nc.sync.dma_start(
                    out=pm_d.ap()[bb * P:(bb + nb) * P, :].rearrange(
                        "(b p) c -> p b c", p=P),
                    in_=tabs3[:, :nb, :])
                if qn > 0:
                    nc.sync.dma_start(
                        out=q_d.ap()[bb * P:(bb + qn) * P, :].rearrange(
                            "(b p) c -> p b c", p=P),
                        in_=qs3[:, :qn, :])

        # ------------- phase A2: local M/SELF tables in f32 -------------
        with tc.tile_pool(name="pa2", bufs=3) as pa2, \
             tc.tile_pool(name="pa2ps", bufs=3, space="PSUM") as pa2ps:
            for bb in range(0, NB_SL, AB):
                nb = min(AB, NB_SL - bb)
                eb2 = pa2.tile([P, AB * D], dt.float32, tag="eb2")
                nc.sync.dma_start(
                    out=eb2[:, :nb * D],
                    in_=emb32[bb * P:(bb + nb) * P, :].rearrange(
                        "(b p) d -> p b d", p=P))
                ms = pa2.tile([P, AB * 256], dt.float32, tag="ms")
                ms3 = ms[:].rearrange("p (b c) -> p b c", c=256)
                for b in range(nb):
                    eT_ps = pa2ps.tile([P, P], dt.float32, tag="eT2")
                    nc.tensor.transpose(out=eT_ps[:],
                                        in_=eb2[:, b * D:(b + 1) * D],
                                        identity=ident32[:])
                    eT = pa2.tile([P, P], dt.float32, tag="eT2s")
                    nc.scalar.copy(out=eT[:], in_=eT_ps[:])
                    mm2 = pa2ps.tile([P, 256], dt.float32, tag="mm2")
                    nc.tensor.matmul(out=mm2[:], lhsT=eT[:], rhs=wms_t[:],
                                     start=True, stop=True)
                    if b % 2 == 0:
                        nc.scalar.copy(out=ms3[:, b, :], in_=mm2[:])
                    else:
                        nc.vector.tensor_copy(out=ms3[:, b, :], in_=mm2[:])
                nc.sync.dma_start(
                    out=m32_d.ap()[bb * P:(bb + nb) * P, :].rearrange(
                        "(b p) c -> p b c", p=P),
                    in_=ms3[:, :nb, 0:D])
                nc.sync.dma_start(
                    out=s32_d.ap()[bb * P:(bb + nb) * P, :].rearrange(
                        "(b p) c -> p b c", p=P),
                    in_=ms3[:, :nb, D:256])

        # ---------------- phase B: edges ----------------
        with tc.tile_pool(name="pb", bufs=2) as pb:
            for st in range(NST):
                dq = pb.tile([P, W // 16], dt.int16, tag="dq")
                nc.sync.dma_start(out=dq[:], in_=erdq_d[st])
                si = pb.tile([P, W // 16], dt.int16, tag="si")
                nc.sync.dma_start(out=si[:], in_=srci_d[st])

                pmt = pb.tile([P, KC * 256], dt.bfloat16, tag="pmt")
                pm3 = pmt[:].rearrange("p (k d) -> p k d", d=256)
                qt = pb.tile([P, KC * D], dt.bfloat16, tag="qt")
                q3 = qt[:].rearrange("p (k d) -> p k d", d=D)
                wc = WQ // P   # 8 payload cols per quarter-run
                for q in range(4):
                    nc.gpsimd.dma_gather(
                        out_ap=pm3[:, q * wc:(q + 1) * wc, :],
                        in_ap=pm_d.ap()[q * QR:(q + 1) * QR, :],
                        idxs_ap=dq[:, q * (WQ // 16):(q + 1) * (WQ // 16)],
                        num_idxs=WQ, num_idxs_reg=WQ, elem_size=256,
                        queue_num=q)
                    nc.gpsimd.dma_gather(
                        out_ap=q3[:, q * wc:(q + 1) * wc, :],
                        in_ap=q_d.ap(),
                        idxs_ap=si[:, q * (WQ // 16):(q + 1) * (WQ // 16)],
                        num_idxs=WQ, num_idxs_reg=WQ, elem_size=D,
                        queue_num=(q + 1) % 4)

                nc.vector.tensor_add(out=q3, in0=pm3[:, :, 0:D], in1=q3)
                nc.scalar.activation(out=q3, in_=q3,
                                     func=mybir.ActivationFunctionType.Tanh)
                w03 = w0_t[:].rearrange("p (o d) -> p o d", o=1).to_broadcast(
                    [P, KC, D])
                nc.vector.tensor_mul(out=q3, in0=q3, in1=w03)
                et = pb.tile([P, KC], dt.float32, tag="et")
                nc.vector.reduce_sum(out=et[:], in_=q3, axis=mybir.AxisListType.X)
                gt = pb.tile([P, KC], dt.float32, tag="gt")
                nc.scalar.activation(out=gt[:], in_=et[:],
                                     func=mybir.ActivationFunctionType.Exp,
                                     bias=b0_t[:])
                gtb = pb.tile([P, KC], dt.bfloat16, tag="gtb")
                nc.vector.tensor_copy(out=gtb[:], in_=gt[:])
                uin = pb.tile([P, KC * 129], dt.float32, tag="uin")
                u3 = uin[:].rearrange("p (k d) -> p k d", d=129)
                g3 = gtb[:].rearrange("p (k o) -> p k o", o=1).to_broadcast(
                    [P, KC, D])
                nc.vector.tensor_mul(out=u3[:, :, 0:D], in0=pm3[:, :, D:256],
                                     in1=g3)
                nc.vector.tensor_copy(out=u3[:, :, D:D + 1],
                                      in_=gt[:].rearrange("p (k o) -> p k o", o=1))
                for q in range(4):
                    nc.gpsimd.dma_scatter_add(
                        out_ap=u_d.ap()[:, 0:129],
                        in_ap=u3[:, q * wc:(q + 1) * wc, :],
                        idxs_ap=si[:, q * (WQ // 16):(q + 1) * (WQ // 16)],
                        num_idxs=WQ, num_idxs_reg=WQ, elem_size=129,
                        elem_step=UW, queue_num=(q + 2) % 4)

        # ---------------- phase C: combine ----------------
        with tc.tile_pool(name="pc", bufs=3) as pc:
            for bb in range(0, NB_SL, CB):
                ut = pc.tile([P, CB * 129], dt.float32, tag="ut")
                nc.sync.dma_start(
                    out=ut[:],
                    in_=u_d.ap()[bb * P:(bb + CB) * P, 0:129].rearrange(
                        "(b p) c -> p b c", p=P))
                mt = pc.tile([P, CB * D], dt.float32, tag="mt")
                nc.sync.dma_start(
                    out=mt[:],
                    in_=m32_d.ap()[bb * P:(bb + CB) * P, :].rearrange(
                        "(b p) c -> p b c", p=P))
                sf = pc.tile([P, CB * D], dt.float32, tag="sf")
                nc.sync.dma_start(
                    out=sf[:],
                    in_=s32_d.ap()[bb * P:(bb + CB) * P, :].rearrange(
                        "(b p) c -> p b c", p=P))
                ut3 = ut[:].rearrange("p (b c) -> p b c", c=129)
                mt3 = mt[:].rearrange("p (b c) -> p b c", c=D)
                sf3 = sf[:].rearrange("p (b c) -> p b c", c=D)
                ob = pc.tile([P, CB * D], dt.float32, tag="ob")
                ob3 = ob[:].rearrange("p (b c) -> p b c", c=D)
                for b in range(CB):
                    blk = bb + b
                    zp = pc.tile([P, 1], dt.float32, tag="zp")
                    nc.vector.tensor_scalar_add(zp[:], ut3[:, b, D:D + 1], 1e-9)
                    rr = pc.tile([P, 1], dt.float32, tag="rr")
                    nc.vector.reciprocal(rr[:], zp[:])
                    zr = pc.tile([P, 1], dt.float32, tag="zr")
                    nc.vector.tensor_mul(out=zr[:], in0=ut3[:, b, D:D + 1],
                                         in1=rr[:])
                    s1 = pc.tile([P, D], dt.float32, tag="s1")
                    nc.vector.tensor_scalar_mul(s1[:], ut3[:, b, 0:D], rr[:])
                    nc.vector.scalar_tensor_tensor(
                        out=s1[:], in0=bn_t[:], scalar=zr[:], in1=s1[:],
                        op0=mybir.AluOpType.mult, op1=mybir.AluOpType.add)
                    t1 = pc.tile([P, D], dt.float32, tag="t1")
                    nc.vector.tensor_add(out=t1[:], in0=mt3[:, b, :], in1=bn_t[:])
                    nc.vector.scalar_tensor_tensor(
                        out=s1[:], in0=t1[:], scalar=sw_t[:, blk:blk + 1],
                        in1=s1[:], op0=mybir.AluOpType.mult,
                        op1=mybir.AluOpType.add)
                    nc.vector.tensor_add(out=s1[:], in0=s1[:], in1=sf3[:, b, :])
                    nc.vector.tensor_add(out=s1[:], in0=s1[:], in1=bs_t[:])
                    nc.scalar.activation(out=ob3[:, b, :], in_=s1[:],
                                         func=mybir.ActivationFunctionType.Tanh)
                nc.sync.dma_start(
                    out=out.ap()[bb * P:(bb + CB) * P, :].rearrange(
                        "(b p) c -> p b c", p=P),
                    in_=ob3)

    nc.compile()
    _CACHE["nc"] = nc
    return nc


def _wrap16(flat, width):
    """Pack flat int16 idx run into [128, width]: idx j at (j%16, j//16),
    replicated across the 8 16-partition groups."""
    n = len(flat)
    assert n % 16 == 0 and n // 16 <= width, (n, width)
    a = np.zeros((16, width), dtype=np.int16)
    a[:, :n // 16] = flat.reshape(-1, 16).T
    return np.tile(a, (8, 1))


def kernel(node_emb, er_src, er_dst, ee_src, ee_dst, ee_weight,
           W_attn_w, W_attn_b, w0_w, w0_b, W_self_w, W_self_b,
           W_neigh_w, W_neigh_b, **_):
    node_emb = np.asarray(node_emb, np.float32)
    er_src = np.asarray(er_src).astype(np.int64)
    er_dst = np.asarray(er_dst).astype(np.int64)
    ee_src = np.asarray(ee_src).astype(np.int64)
    ee_weight = np.asarray(ee_weight, np.float32)

    wallb = np.concatenate([
        np.asarray(W_attn_w, np.float32)[:D],
        np.asarray(W_neigh_w, np.float32),
        np.asarray(W_attn_w, np.float32)[D:],
        np.asarray(W_self_w, np.float32)], axis=1).astype(bf16)
    qbias = np.broadcast_to(np.asarray(W_attn_b, np.float32), (P, D)).copy()
    bnrep = np.broadcast_to(np.asarray(W_neigh_b, np.float32), (P, D)).copy()
    bsrep = np.broadcast_to(np.asarray(W_self_b, np.float32), (P, D)).copy()
    w0rep = np.broadcast_to(np.asarray(w0_w, np.float32), (P, D)).astype(bf16)
    b0c = np.full((P, 1), float(np.asarray(w0_b)), np.float32)
    wallms = np.concatenate([np.asarray(W_neigh_w, np.float32),
                             np.asarray(W_self_w, np.float32)], axis=1)

    in_maps = []
    for c in range(NC):
        lo = c * SL
        emb_rot32 = np.concatenate([
            np.roll(node_emb, -lo, axis=0),
            np.zeros((NPAD - N, D), np.float32)], axis=0)
        emb_rot = emb_rot32.astype(bf16)

        sel = (er_src >= lo) & (er_src < lo + SL)
        s_loc = (er_src[sel] - lo).astype(np.int32)
        d_rot = ((er_dst[sel] - lo) % N).astype(np.int32)
        o = np.argsort(s_loc, kind="stable")
        s_loc, d_rot = s_loc[o], d_rot[o]
        deg = np.bincount(s_loc, minlength=SL).max()
        assert deg <= NST, deg

        erdq = np.zeros((NST, P, W // 16), np.int16)
        srci = np.zeros((NST, P, W // 16), np.int16)
        i = np.arange(len(s_loc))
        for st in range(NST):
            g = i[i % NST == st]
            dg, sg = d_rot[g], s_loc[g]
            qid = dg // QR
            dflat = np.zeros(W, np.int16)
            sflat = np.full(W, TRASH, np.int16)
            for q in range(4):
                m = qid == q
                cnt = int(m.sum())
                assert cnt <= WQ, (st, q, cnt)
                dflat[q * WQ:q * WQ + cnt] = (dg[m] - q * QR).astype(np.int16)
                sflat[q * WQ:q * WQ + cnt] = sg[m].astype(np.int16)
            erdq[st] = _wrap16(dflat, W // 16)
            srci[st] = _wrap16(sflat, W // 16)

        esel = (ee_src >= lo) & (ee_src < lo + SL)
        es_loc = (ee_src[esel] - lo).astype(np.int64)
        sw = np.bincount(es_loc, weights=ee_weight[esel],
                         minlength=NB_SL * P).astype(np.float32)
        swt = sw.reshape(NB_SL, P).T.copy()

        in_maps.append({
            "embb": emb_rot, "wallb": wallb, "qbias": qbias, "bnrep": bnrep,
            "bsrep": bsrep, "w0r": w0rep, "b0c": b0c, "swt": swt,
            "emb32": emb_rot32[:NB_SL * P], "wallms": wallms,
            "erdq": erdq, "srci": srci,
        })

    nc = _build()
    res = run_bass_kernel_spmd(nc, in_maps, core_ids=list(range(NC)),
                               tmpdir=os.environ.get("BASS_TMPDIR"))
    globals()["LAST"] = res
    return np.concatenate([res.results[c]["out"][:SL] for c in range(NC)], axis=0)
